# revision 15
# baseline (speedup 1.0000x reference)
"""MixHop GNN kernel for one TRN2 chip (8 NeuronCores), Bass/Tile.

Math (matches the reference exactly):
    row/col = edge_index with self loops appended
    deg[t]  = #edges with col==t            (host: integer bincount)
    dinv    = 1/sqrt(deg)                   (device: sqrt + reciprocal)
    h1[t]   = dinv_t * (sum_{s->t} dinv_s * x_s  + dinv_t * x_t)
    h2[t]   = dinv_t * (sum_{s->t} dinv_s * h1_s + dinv_t * h1_t)
    out = relu(concat(x@W0+b0, h1@W1+b1, h2@W2+b2)) @ Wout + bout

Sharding: core c owns target nodes [c*N/8, (c+1)*N/8). Edges (self loops
excluded -- those enter via the z_stage add, since the needed value is
resident) are bucketed by target into windows of 128 consecutive
targets, split by source (< 32768 vs >=, the int16 limit of dma_gather),
each part padded to blocks of 128 (uniform across cores -> one SPMD
program). Per window: dma_gather pulls source rows straight from the
raw x table (hop 1) / the AllGathered h1 table (hop 2) -- the source-
side norm factor is folded into the selection matrix S built with ONE
fused DVE op: S = (iota == tl) * dinv_src,
and a PE matmul S.T @ G accumulates the scaled segment-sum in PSUM.
Gather descriptor generation is the bottleneck engine (GpSimd SWDGE,
~8.4 ns/idx on one queue), so consecutive gather calls alternate
between SWDGE queues 0/1, which overlap generation (~5.9 ns/idx).
Only ONE collective remains (AllGather of h1); the hop-1 table is the
raw x input. The dense head is interleaved into the hop-2 window loop.
"""
import numpy as np
import ml_dtypes

N = 50000
F = 128
NCORE = 8
NPC = N // NCORE          # 6250 nodes per core
WIN = 128                 # targets per window
NWIN = (NPC + WIN - 1) // WIN   # 49 (48 full + 1 partial of 106)
PER_HOP = 64
OUT = 64
XA = 3200                 # per-core rows in table half A (25 windows)
XB = NPC - XA             # 3050 rows in half B
NA = NCORE * XA           # 25600 rows in table A
NB = NCORE * XB           # 24400 rows in table B
MAXBLK = 8                # max 1024 idxs per dma_gather call
PAD_TL = 300.0            # dummy-edge tl: matches no iota value -> zero S row


MAXG = 64                 # 16-idx groups per call (64 -> 1024 idxs)


def _chunks(ng):
    """Split ng 16-idx groups into calls <= MAXG; all but the last call
    must cover a multiple of 128 idxs (8 groups) so each call starts at a
    128-slot boundary of the gather output."""
    if ng <= 0:
        return []
    k = (ng + MAXG - 1) // MAXG
    out = []
    left = ng
    for i in range(k - 1):
        c = min(MAXG, max(8, ((left // (k - i)) + 7) // 8 * 8))
        out.append(c)
        left -= c
    out.append(left)
    return out


def _preprocess(edge_index):
    """Bucket edges by (core, target-window, source-half); pad uniformly.

    Returns (NBL, NBH, per_core list of dicts with idx16, tl_t, ds_t
    (dinv_src per slot), deg_t).
    """
    row = np.asarray(edge_index[0], dtype=np.int64)
    col = np.asarray(edge_index[1], dtype=np.int64)
    deg = (np.bincount(col, minlength=N) + 1).astype(np.float64)
    dinv = (1.0 / np.sqrt(deg)).astype(np.float32)

    cores = []
    for c in range(NCORE):
        lo, hi = c * NPC, (c + 1) * NPC
        sel = (col >= lo) & (col < hi)
        r = row[sel]
        t = col[sel] - lo
        order = np.argsort(t, kind="stable")
        r, t = r[order], t[order]
        rc, ri = r // NPC, r % NPC
        is_lo = ri < XA
        ra = rc * XA + ri              # row in table A
        rb = rc * XB + (ri - XA)       # row in table B
        parts = []
        for w in range(NWIN):
            wsel = (t // WIN) == w
            rl = ra[wsel & is_lo]
            tl_l = (t[wsel & is_lo] % WIN).astype(np.float32)
            dl = r[wsel & is_lo]
            rh = rb[wsel & ~is_lo]
            tl_h = (t[wsel & ~is_lo] % WIN).astype(np.float32)
            dh = r[wsel & ~is_lo]
            parts.append((rl, tl_l, dl, rh, tl_h, dh))
        cores.append(parts)

    # real max counts (over cores), rounded to 16 (idx wrap granularity)
    GLs = tuple(int(-(-max(len(cores[c][w][0]) for c in range(NCORE)) // 16))
                for w in range(NWIN))
    GHs = tuple(int(-(-max(len(cores[c][w][3]) for c in range(NCORE)) // 16))
                for w in range(NWIN))
    NBLs = tuple((g * 16 + 127) // 128 for g in GLs)
    NBHs = tuple((g * 16 + 127) // 128 for g in GHs)
    IDXOFF = np.concatenate(
        [[0], np.cumsum([gl + gh for gl, gh in zip(GLs, GHs)])]).astype(int)
    TLOFF = np.concatenate(
        [[0], np.cumsum([nl + nh for nl, nh in zip(NBLs, NBHs)])]).astype(int)
    out = []
    for ci, parts in enumerate(cores):
        idx16 = np.zeros((128, int(IDXOFF[-1])), dtype=np.int16)
        tl_t = np.full((128, int(TLOFF[-1])), PAD_TL, dtype=np.float32)
        ds_t = np.zeros((128, int(TLOFF[-1])), dtype=np.float32)
        for w, (rl, tl_l, dl, rh, tl_h, dh) in enumerate(parts):
            nbl_w, nbh_w = NBLs[w], NBHs[w]
            nbt_w = nbl_w + nbh_w
            gl_w, gh_w = GLs[w], GHs[w]
            lo_chunks, hi_chunks = _chunks(gl_w), _chunks(gh_w)
            ilo = np.zeros(gl_w * 16, dtype=np.int64)
            ilo[:len(rl)] = rl
            ihi = np.zeros(gh_w * 16, dtype=np.int64)
            ihi[:len(rh)] = rh
            # tl / dinv_src streams: [lo blocks..., hi blocks...]
            tw = np.full(nbt_w * 128, PAD_TL, dtype=np.float32)
            tw[:len(tl_l)] = tl_l
            tw[nbl_w * 128:nbl_w * 128 + len(tl_h)] = tl_h
            tl_t[:, TLOFF[w]:TLOFF[w + 1]] = tw.reshape(nbt_w, 128).T
            dw = np.zeros(nbt_w * 128, dtype=np.float32)
            dw[:len(dl)] = dinv[dl]
            dw[nbl_w * 128:nbl_w * 128 + len(dh)] = dinv[dh]
            ds_t[:, TLOFF[w]:TLOFF[w + 1]] = dw.reshape(nbt_w, 128).T
            # idx stream: per call, [16-wrap then replicate x8 partitions]
            cols = []
            for chunks, arr in ((lo_chunks, ilo), (hi_chunks, ihi)):
                off = 0
                for cg in chunks:
                    a = arr[off:off + cg * 16].astype(np.int16)
                    cols.append(np.tile(a.reshape(-1, 16).T, (8, 1)))
                    off += cg * 16
            if cols:
                idx16[:, IDXOFF[w]:IDXOFF[w + 1]] = np.concatenate(
                    cols, axis=1)
        lo = ci * NPC
        dpad = np.ones(NWIN * WIN, dtype=np.float32)
        dpad[:NPC] = deg[lo:lo + NPC]
        out.append({"idx16": np.ascontiguousarray(idx16),
                    "tl_t": np.ascontiguousarray(
                        tl_t.astype(ml_dtypes.bfloat16)),
                    "ds_t": np.ascontiguousarray(
                        ds_t.astype(ml_dtypes.bfloat16)),
                    "deg_t": np.ascontiguousarray(
                        dpad.reshape(NWIN, WIN).T.astype(np.float32))})
    return (NBLs, NBHs, GLs, GHs), out


def _build(NBLs, NBHs, GLs, GHs):
    import concourse.bass as bass  # noqa: F401
    import concourse.bacc as bacc
    import concourse.tile as tile
    import concourse.mybir as mybir
    from concourse.masks import make_identity

    dt = mybir.dt
    f32 = dt.float32
    bf16 = dt.bfloat16
    AF = mybir.ActivationFunctionType
    ALU = mybir.AluOpType
    NBTs = [nl + nh for nl, nh in zip(NBLs, NBHs)]
    NBTMAX = max(NBTs)
    IDXOFF = np.concatenate(
        [[0], np.cumsum([gl + gh for gl, gh in zip(GLs, GHs)])]).astype(int)
    TLOFF = np.concatenate([[0], np.cumsum(NBTs)]).astype(int)
    NFULL = (NWIN - 1) * WIN                 # 6144 rows in full windows
    NLAST = NPC - NFULL                      # 106 rows in the partial window

    nc = bacc.Bacc("TRN2", target_bir_lowering=False, debug=False,
                   num_devices=NCORE, num_swdge_queues=2)

    xa_in = nc.dram_tensor("x_a", [NA, F], bf16, kind="ExternalInput")
    xb_in = nc.dram_tensor("x_b", [NB, F], bf16, kind="ExternalInput")
    xo_in = nc.dram_tensor("x_own", [NPC, F], f32, kind="ExternalInput")
    idx_in = nc.dram_tensor("idx16", [128, int(IDXOFF[-1])], dt.int16,
                            kind="ExternalInput")
    tl_in = nc.dram_tensor("tl_t", [128, int(TLOFF[-1])], bf16,
                           kind="ExternalInput")
    ds_in = nc.dram_tensor("ds_t", [128, int(TLOFF[-1])], bf16,
                           kind="ExternalInput")
    deg_in = nc.dram_tensor("deg_t", [128, NWIN], f32, kind="ExternalInput")
    iot_in = nc.dram_tensor("iot", [128, 128], bf16, kind="ExternalInput")
    w0_in = nc.dram_tensor("w0", [F, PER_HOP], f32, kind="ExternalInput")
    w1_in = nc.dram_tensor("w1", [F, PER_HOP], f32, kind="ExternalInput")
    w2_in = nc.dram_tensor("w2", [F, PER_HOP], f32, kind="ExternalInput")
    wo_in = nc.dram_tensor("wout", [3 * PER_HOP, OUT], f32,
                           kind="ExternalInput")
    b0_in = nc.dram_tensor("b0", [PER_HOP, 1], f32, kind="ExternalInput")
    b1_in = nc.dram_tensor("b1", [PER_HOP, 1], f32, kind="ExternalInput")
    b2_in = nc.dram_tensor("b2", [PER_HOP, 1], f32, kind="ExternalInput")
    bo_in = nc.dram_tensor("bout", [OUT, 1], f32, kind="ExternalInput")
    out_t = nc.dram_tensor("out_t", [OUT, NPC], f32, kind="ExternalOutput")

    h1b = nc.dram_tensor("h1b", [NPC, F], bf16)
    h1fa = nc.dram_tensor("h1fa", [NA, F], bf16, addr_space="Shared")
    h1fb = nc.dram_tensor("h1fb", [NB, F], bf16, addr_space="Shared")

    def ts(w):
        return slice(w * WIN, (w + 1) * WIN)

    with tile.TileContext(nc) as tc:
        with (
            tc.tile_pool(name="persist", bufs=1) as pp,
            tc.tile_pool(name="gbuf", bufs=4) as gp,
            tc.tile_pool(name="work", bufs=4) as wp,
            tc.tile_pool(name="psum_y", bufs=2, space="PSUM") as psy,
            tc.tile_pool(name="psum_t", bufs=2, space="PSUM") as pst,
            tc.tile_pool(name="psum_d", bufs=2, space="PSUM") as psd,
        ):
            # ---- persistent loads ----
            idx_sb = pp.tile([128, int(IDXOFF[-1])], dt.int16)
            nc.sync.dma_start(out=idx_sb[:], in_=idx_in[:])
            tl_sb = pp.tile([128, int(TLOFF[-1])], bf16)
            nc.sync.dma_start(out=tl_sb[:], in_=tl_in[:])
            ds_sb = pp.tile([128, int(TLOFF[-1])], bf16)
            nc.sync.dma_start(out=ds_sb[:], in_=ds_in[:])
            iot_sb = pp.tile([128, 128], bf16)
            nc.sync.dma_start(out=iot_sb[:], in_=iot_in[:])
            deg_sb = pp.tile([128, NWIN], f32)
            nc.sync.dma_start(out=deg_sb[:], in_=deg_in[:])
            w0_sb = pp.tile([F, PER_HOP], f32)
            nc.sync.dma_start(out=w0_sb[:], in_=w0_in[:])
            w1_sb = pp.tile([F, PER_HOP], f32)
            nc.sync.dma_start(out=w1_sb[:], in_=w1_in[:])
            w2_sb = pp.tile([F, PER_HOP], f32)
            nc.sync.dma_start(out=w2_sb[:], in_=w2_in[:])
            wo_sb = []
            for k in range(3):
                t = pp.tile([PER_HOP, OUT], f32, tag=f"wo{k}")
                nc.sync.dma_start(
                    out=t[:], in_=wo_in.ap()[k * PER_HOP:(k + 1) * PER_HOP, :])
                wo_sb.append(t)
            b_sb = []
            for k, bin_ in enumerate((b0_in, b1_in, b2_in)):
                t = pp.tile([PER_HOP, 1], f32, tag=f"b{k}")
                nc.sync.dma_start(out=t[:], in_=bin_[:])
                b_sb.append(t)
            bo_sb = pp.tile([OUT, 1], f32)
            nc.sync.dma_start(out=bo_sb[:], in_=bo_in[:])
            ident = pp.tile([128, 128], f32)
            make_identity(nc, ident[:])
            iot_wide = pp.tile([128, NBTMAX * 128], bf16)
            for j in range(NBTMAX):
                nc.vector.tensor_copy(out=iot_wide[:, j * 128:(j + 1) * 128],
                                      in_=iot_sb[:])

            # dinv = 1/sqrt(deg); dinv2 = dinv^2  (both [128, NWIN])
            sq = pp.tile([128, NWIN], f32)
            nc.scalar.activation(out=sq[:], in_=deg_sb[:], func=AF.Sqrt)
            dinv = pp.tile([128, NWIN], f32)
            nc.vector.reciprocal(out=dinv[:], in_=sq[:])
            dinv2 = pp.tile([128, NWIN], f32)
            nc.vector.tensor_tensor(out=dinv2[:], in0=dinv[:], in1=dinv[:],
                                    op=ALU.mult)

            # ---- load x (window-major: [p, w*128+f] = x[w*128+p, f]) ----
            x_sb = pp.tile([128, NWIN * WIN], f32)
            nc.vector.memset(x_sb[:, (NWIN - 1) * WIN:], 0.0)
            nc.sync.dma_start(
                out=x_sb[:].rearrange("p (w f) -> p w f", f=F)[:, 0:NWIN - 1, :],
                in_=xo_in.ap()[0:NFULL, :].rearrange("(w p) f -> p w f", p=128),
            )
            nc.sync.dma_start(
                out=x_sb[0:NLAST, (NWIN - 1) * WIN:],
                in_=xo_in.ap()[NFULL:NPC, :],
            )

            # z_stage: hop1 self term dinv*x; overwritten to dinv^2*h1 later
            z_stage = pp.tile([128, NWIN * WIN], f32)
            for w in range(NWIN):
                nc.vector.tensor_scalar_mul(
                    out=z_stage[:, ts(w)], in0=x_sb[:, ts(w)],
                    scalar1=dinv[:, w:w + 1])

            h1_sb = pp.tile([128, NWIN * WIN], f32)
            qctr = [0]

            def head(w, h2_sb):
                relus = []
                for k, (h_sb, wk_sb) in enumerate(
                        ((x_sb, w0_sb), (h1_sb, w1_sb), (h2_sb, w2_sb))):
                    tp = pst.tile([128, 128], f32, tag="tp")
                    nc.tensor.transpose(out=tp[:], in_=h_sb[:, ts(w)],
                                        identity=ident[:])
                    hT = wp.tile([128, 128], f32, tag="hT")
                    nc.vector.tensor_copy(out=hT[:], in_=tp[:])
                    cps = psd.tile([PER_HOP, 128], f32, tag="cps")
                    nc.tensor.matmul(out=cps[:], lhsT=wk_sb[:], rhs=hT[:],
                                     start=True, stop=True)
                    rk = wp.tile([PER_HOP, 128], f32, tag=f"r{k}")
                    nc.scalar.activation(out=rk[:], in_=cps[:], func=AF.Relu,
                                         bias=b_sb[k][:])
                    relus.append(rk)
                ops = psd.tile([OUT, 128], f32, tag="ops")
                for k in range(3):
                    nc.tensor.matmul(out=ops[:], lhsT=wo_sb[k][:],
                                     rhs=relus[k][:],
                                     start=(k == 0), stop=(k == 2))
                ow = wp.tile([OUT, 128], f32, tag="ow")
                nc.scalar.activation(out=ow[:], in_=ops[:],
                                     func=AF.Identity, bias=bo_sb[:])
                lim = min(NPC, (w + 1) * WIN) - w * WIN
                nc.sync.dma_start(out=out_t.ap()[:, w * WIN:w * WIN + lim],
                                  in_=ow[:, 0:lim])

            def prop(tableA, tableB, dss, h_out, hop2):
                """One propagation sweep; hop2 also runs the head."""
                first4 = not hop2
                for w in range(NWIN):
                    NBT = NBTs[w]
                    g = gp.tile([128, NBTMAX * F], bf16, tag="g")
                    if first4 and w < 4:
                        # pool buffers start uninitialized; un-gathered tail
                        # slots must hold finite bf16 (S rows are 0 there)
                        nc.vector.memset(g[:], 0.0)
                    icol = int(IDXOFF[w])
                    blk = 0
                    for part, cgs, nbp in ((0, _chunks(GLs[w]), NBLs[w]),
                                           (1, _chunks(GHs[w]), NBHs[w])):
                        src = (tableA.ap()[:, :] if part == 0
                               else tableB.ap()[:, :])
                        done = 0          # idxs done within this part
                        for cg in cgs:
                            # chunk starts at a 128-slot boundary of the part
                            assert done % 128 == 0
                            nb_call = (cg * 16 + 127) // 128
                            b0 = blk + done // 128
                            nc.gpsimd.dma_gather(
                                out_ap=g[:, b0 * F:
                                         (b0 + nb_call) * F].rearrange(
                                    "p (b f) -> p b f", f=F),
                                in_ap=src,
                                idxs_ap=idx_sb[:, icol:icol + cg],
                                num_idxs=cg * 16, num_idxs_reg=cg * 16,
                                elem_size=F, queue_num=qctr[0] % 2)
                            qctr[0] += 1
                            icol += cg
                            done += cg * 16
                        blk += nbp
                    ps = psy.tile([128, F], f32)
                    c0, c1 = int(TLOFF[w]), int(TLOFF[w + 1])
                    sw = wp.tile([128, NBTMAX * 128], bf16, tag="s")
                    nc.vector.tensor_tensor(
                        out=sw[:, 0:NBT * 128].rearrange(
                            "p (b t) -> p b t", t=128),
                        in0=iot_wide[:, 0:NBT * 128].rearrange(
                            "p (b t) -> p b t", t=128),
                        in1=tl_sb[:, c0:c1].rearrange(
                            "p (b o) -> p b o", o=1).broadcast_to(
                            [128, NBT, 128]),
                        op=ALU.is_equal)
                    nc.vector.tensor_tensor(
                        out=sw[:, 0:NBT * 128].rearrange(
                            "p (b t) -> p b t", t=128),
                        in0=sw[:, 0:NBT * 128].rearrange(
                            "p (b t) -> p b t", t=128),
                        in1=dss[:, c0:c1].rearrange(
                            "p (b o) -> p b o", o=1).broadcast_to(
                            [128, NBT, 128]),
                        op=ALU.mult)
                    for j in range(NBT):
                        nc.tensor.matmul(
                            out=ps[:], lhsT=sw[:, j * 128:(j + 1) * 128],
                            rhs=g[:, j * F:(j + 1) * F],
                            start=(j == 0), stop=(j == NBT - 1))
                    # self loop term (z_stage), then h = dinv * ya
                    ya = wp.tile([128, F], f32, tag="ya")
                    nc.vector.tensor_tensor(
                        out=ya[:], in0=ps[:], in1=z_stage[:, ts(w)],
                        op=ALU.add)
                    nc.vector.tensor_scalar_mul(
                        out=h_out[:, ts(w)], in0=ya[:],
                        scalar1=dinv[:, w:w + 1])
                    if not hop2:
                        # stage hop-2 self term dinv*h1 (prop applies the
                        # remaining dinv_t); bounce h1 window
                        nc.vector.tensor_scalar_mul(
                            out=z_stage[:, ts(w)], in0=h_out[:, ts(w)],
                            scalar1=dinv[:, w:w + 1])
                        hb = wp.tile([128, F], bf16, tag="hb")
                        nc.vector.tensor_copy(out=hb[:], in_=h_out[:, ts(w)])
                        lim = min(NPC, (w + 1) * WIN) - w * WIN
                        nc.sync.dma_start(
                            out=h1b.ap()[w * WIN:w * WIN + lim, :],
                            in_=hb[0:lim, :])
                        if w == XA // WIN - 1:
                            # rows [0, XA) complete on every core: exchange
                            # table half A under the remaining hop-1 windows
                            nc.gpsimd.collective_compute(
                                "AllGather", ALU.bypass,
                                replica_groups=[list(range(NCORE))],
                                ins=[h1b.ap()[0:XA, :]], outs=[h1fa[:]])
                    else:
                        head(w, h_out)

            prop(xa_in, xb_in, ds_sb, h1_sb, hop2=False)
            nc.gpsimd.collective_compute(
                "AllGather", ALU.bypass,
                replica_groups=[list(range(NCORE))],
                ins=[h1b.ap()[XA:NPC, :]], outs=[h1fb[:]])

            # hop 2: same per-edge norm dinv_src * dinv_tgt (h1 is raw)
            h2_sb = pp.tile([128, NWIN * WIN], f32)
            prop(h1fa, h1fb, ds_sb, h2_sb, hop2=True)

    nc.compile()
    return nc


_CACHE = {}


def _get_nc(key):
    if key not in _CACHE:
        _CACHE[key] = _build(*key)
    return _CACHE[key]


def make_in_maps(x, pc, W0, b0, W1, b1, W2, b2, Wout, bout):
    iot = np.broadcast_to(
        np.arange(128, dtype=np.float32), (128, 128)).astype(ml_dtypes.bfloat16)
    x = np.ascontiguousarray(np.asarray(x, dtype=np.float32))
    xb16 = x.astype(ml_dtypes.bfloat16)
    xa = np.concatenate([xb16[c * NPC:c * NPC + XA] for c in range(NCORE)])
    xbt = np.concatenate([xb16[c * NPC + XA:(c + 1) * NPC]
                          for c in range(NCORE)])
    common = {
        "iot": iot,
        "x_a": np.ascontiguousarray(xa),
        "x_b": np.ascontiguousarray(xbt),
        "w0": np.asarray(W0, dtype=np.float32),
        "w1": np.asarray(W1, dtype=np.float32),
        "w2": np.asarray(W2, dtype=np.float32),
        "wout": np.asarray(Wout, dtype=np.float32),
        "b0": np.asarray(b0, dtype=np.float32).reshape(PER_HOP, 1),
        "b1": np.asarray(b1, dtype=np.float32).reshape(PER_HOP, 1),
        "b2": np.asarray(b2, dtype=np.float32).reshape(PER_HOP, 1),
        "bout": np.asarray(bout, dtype=np.float32).reshape(OUT, 1),
    }
    in_maps = []
    for c in range(NCORE):
        m = dict(common)
        m.update(pc[c])
        m["x_own"] = np.ascontiguousarray(x[c * NPC:(c + 1) * NPC])
        in_maps.append(m)
    return in_maps


def run(inputs, trace=False):
    from concourse.bass_utils import run_bass_kernel_spmd

    key, pc = _preprocess(np.asarray(inputs["edge_index"]))
    nc = _get_nc(key)
    in_maps = make_in_maps(
        inputs["x"], pc, inputs["W0"], inputs["b0"], inputs["W1"],
        inputs["b1"], inputs["W2"], inputs["b2"], inputs["Wout"],
        inputs["bout"])
    res = run_bass_kernel_spmd(nc, in_maps, core_ids=list(range(NCORE)),
                               trace=trace)
    out = np.empty((N, OUT), dtype=np.float32)
    for c in range(NCORE):
        out[c * NPC:(c + 1) * NPC] = np.asarray(res.results[c]["out_t"]).T
    return out, res


def kernel(x, edge_index, W0, b0, W1, b1, W2, b2, Wout, bout):
    out, _ = run({"x": x, "edge_index": edge_index, "W0": W0, "b0": b0,
                  "W1": W1, "b1": b1, "W2": W2, "b2": b2,
                  "Wout": Wout, "bout": bout})
    return out


# revision 16
# speedup vs baseline: 1.0548x; 1.0548x over previous
"""MixHop GNN kernel for one TRN2 chip (8 NeuronCores), Bass/Tile.

Math (matches the reference exactly):
    row/col = edge_index with self loops appended
    deg[t]  = #edges with col==t            (host: integer bincount)
    dinv    = 1/sqrt(deg)                   (device: sqrt + reciprocal)
    h1[t]   = dinv_t * (sum_{s->t} dinv_s * x_s  + dinv_t * x_t)
    h2[t]   = dinv_t * (sum_{s->t} dinv_s * h1_s + dinv_t * h1_t)
    out = relu(concat(x@W0+b0, h1@W1+b1, h2@W2+b2)) @ Wout + bout

Sharding: core c owns target nodes [c*N/8, (c+1)*N/8). Edges (self loops
excluded -- those enter via the z_stage add, since the needed value is
resident) are bucketed by target into windows of 128 consecutive
targets, split by source (< 32768 vs >=, the int16 limit of dma_gather),
each part padded to blocks of 128 (uniform across cores -> one SPMD
program). Per window: dma_gather pulls source rows straight from the
raw x table (hop 1) / the AllGathered h1 table (hop 2) -- the source-
side norm factor is folded into the selection matrix S built with ONE
fused DVE op: S = (iota == tl) * dinv_src,
and a PE matmul S.T @ G accumulates the scaled segment-sum in PSUM.
Gather descriptor generation is the bottleneck engine (GpSimd SWDGE,
~8.4 ns/idx on one queue), so consecutive gather calls alternate
between SWDGE queues 0/1, which overlap generation (~5.9 ns/idx).
Only ONE collective remains (AllGather of h1); the hop-1 table is the
raw x input. The dense head is interleaved into the hop-2 window loop.
"""
import numpy as np
import ml_dtypes

N = 50000
F = 128
NCORE = 8
NPC = N // NCORE          # 6250 nodes per core
WIN = 128                 # targets per window
NWIN = (NPC + WIN - 1) // WIN   # 49 (48 full + 1 partial of 106)
PER_HOP = 64
OUT = 64
SPLIT = 32768             # int16 index limit for dma_gather tables
MAXBLK = 8                # max 1024 idxs per dma_gather call
PAD_TL = 300.0            # dummy-edge tl: matches no iota value -> zero S row


MAXG = 64                 # 16-idx groups per call (64 -> 1024 idxs)


def _chunks(ng):
    """Split ng 16-idx groups into calls <= MAXG; all but the last call
    must cover a multiple of 128 idxs (8 groups) so each call starts at a
    128-slot boundary of the gather output."""
    if ng <= 0:
        return []
    k = (ng + MAXG - 1) // MAXG
    out = []
    left = ng
    for i in range(k - 1):
        c = min(MAXG, max(8, ((left // (k - i)) + 7) // 8 * 8))
        out.append(c)
        left -= c
    out.append(left)
    return out


def _preprocess(edge_index):
    """Bucket edges by (core, target-window, source-half); pad uniformly.

    Returns (NBL, NBH, per_core list of dicts with idx16, tl_t, ds_t
    (dinv_src per slot), deg_t).
    """
    row = np.asarray(edge_index[0], dtype=np.int64)
    col = np.asarray(edge_index[1], dtype=np.int64)
    deg = (np.bincount(col, minlength=N) + 1).astype(np.float64)
    dinv = (1.0 / np.sqrt(deg)).astype(np.float32)

    cores = []
    for c in range(NCORE):
        lo, hi = c * NPC, (c + 1) * NPC
        sel = (col >= lo) & (col < hi)
        r = row[sel]
        t = col[sel] - lo
        order = np.argsort(t, kind="stable")
        r, t = r[order], t[order]
        is_lo = r < SPLIT
        ra = r
        rb = r - SPLIT
        parts = []
        for w in range(NWIN):
            wsel = (t // WIN) == w
            rl = ra[wsel & is_lo]
            tl_l = (t[wsel & is_lo] % WIN).astype(np.float32)
            dl = r[wsel & is_lo]
            rh = rb[wsel & ~is_lo]
            tl_h = (t[wsel & ~is_lo] % WIN).astype(np.float32)
            dh = r[wsel & ~is_lo]
            parts.append((rl, tl_l, dl, rh, tl_h, dh))
        cores.append(parts)

    # real max counts (over cores), rounded to 16 (idx wrap granularity)
    GLs = tuple(int(-(-max(len(cores[c][w][0]) for c in range(NCORE)) // 16))
                for w in range(NWIN))
    GHs = tuple(int(-(-max(len(cores[c][w][3]) for c in range(NCORE)) // 16))
                for w in range(NWIN))
    NBLs = tuple((g * 16 + 127) // 128 for g in GLs)
    NBHs = tuple((g * 16 + 127) // 128 for g in GHs)
    IDXOFF = np.concatenate(
        [[0], np.cumsum([gl + gh for gl, gh in zip(GLs, GHs)])]).astype(int)
    TLOFF = np.concatenate(
        [[0], np.cumsum([nl + nh for nl, nh in zip(NBLs, NBHs)])]).astype(int)
    out = []
    for ci, parts in enumerate(cores):
        idx16 = np.zeros((128, int(IDXOFF[-1])), dtype=np.int16)
        tl_t = np.full((128, int(TLOFF[-1])), PAD_TL, dtype=np.float32)
        ds_t = np.zeros((128, int(TLOFF[-1])), dtype=np.float32)
        for w, (rl, tl_l, dl, rh, tl_h, dh) in enumerate(parts):
            nbl_w, nbh_w = NBLs[w], NBHs[w]
            nbt_w = nbl_w + nbh_w
            gl_w, gh_w = GLs[w], GHs[w]
            lo_chunks, hi_chunks = _chunks(gl_w), _chunks(gh_w)
            ilo = np.zeros(gl_w * 16, dtype=np.int64)
            ilo[:len(rl)] = rl
            ihi = np.zeros(gh_w * 16, dtype=np.int64)
            ihi[:len(rh)] = rh
            # tl / dinv_src streams: [lo blocks..., hi blocks...]
            tw = np.full(nbt_w * 128, PAD_TL, dtype=np.float32)
            tw[:len(tl_l)] = tl_l
            tw[nbl_w * 128:nbl_w * 128 + len(tl_h)] = tl_h
            tl_t[:, TLOFF[w]:TLOFF[w + 1]] = tw.reshape(nbt_w, 128).T
            dw = np.zeros(nbt_w * 128, dtype=np.float32)
            dw[:len(dl)] = dinv[dl]
            dw[nbl_w * 128:nbl_w * 128 + len(dh)] = dinv[dh]
            ds_t[:, TLOFF[w]:TLOFF[w + 1]] = dw.reshape(nbt_w, 128).T
            # idx stream: per call, [16-wrap then replicate x8 partitions]
            cols = []
            for chunks, arr in ((lo_chunks, ilo), (hi_chunks, ihi)):
                off = 0
                for cg in chunks:
                    a = arr[off:off + cg * 16].astype(np.int16)
                    cols.append(np.tile(a.reshape(-1, 16).T, (8, 1)))
                    off += cg * 16
            if cols:
                idx16[:, IDXOFF[w]:IDXOFF[w + 1]] = np.concatenate(
                    cols, axis=1)
        lo = ci * NPC
        dpad = np.ones(NWIN * WIN, dtype=np.float32)
        dpad[:NPC] = deg[lo:lo + NPC]
        out.append({"idx16": np.ascontiguousarray(idx16),
                    "tl_t": np.ascontiguousarray(
                        tl_t.astype(ml_dtypes.bfloat16)),
                    "ds_t": np.ascontiguousarray(
                        ds_t.astype(ml_dtypes.bfloat16)),
                    "deg_t": np.ascontiguousarray(
                        dpad.reshape(NWIN, WIN).T.astype(np.float32))})
    return (NBLs, NBHs, GLs, GHs), out


def _build(NBLs, NBHs, GLs, GHs):
    import concourse.bass as bass  # noqa: F401
    import concourse.bacc as bacc
    import concourse.tile as tile
    import concourse.mybir as mybir
    from concourse.masks import make_identity

    dt = mybir.dt
    f32 = dt.float32
    bf16 = dt.bfloat16
    AF = mybir.ActivationFunctionType
    ALU = mybir.AluOpType
    NBTs = [nl + nh for nl, nh in zip(NBLs, NBHs)]
    NBTMAX = max(NBTs)
    IDXOFF = np.concatenate(
        [[0], np.cumsum([gl + gh for gl, gh in zip(GLs, GHs)])]).astype(int)
    TLOFF = np.concatenate([[0], np.cumsum(NBTs)]).astype(int)
    NFULL = (NWIN - 1) * WIN                 # 6144 rows in full windows
    NLAST = NPC - NFULL                      # 106 rows in the partial window

    nc = bacc.Bacc("TRN2", target_bir_lowering=False, debug=False,
                   num_devices=NCORE, num_swdge_queues=2)

    x_in = nc.dram_tensor("x_full", [N, F], bf16, kind="ExternalInput")
    xo_in = nc.dram_tensor("x_own", [NPC, F], f32, kind="ExternalInput")
    idx_in = nc.dram_tensor("idx16", [128, int(IDXOFF[-1])], dt.int16,
                            kind="ExternalInput")
    tl_in = nc.dram_tensor("tl_t", [128, int(TLOFF[-1])], bf16,
                           kind="ExternalInput")
    ds_in = nc.dram_tensor("ds_t", [128, int(TLOFF[-1])], bf16,
                           kind="ExternalInput")
    deg_in = nc.dram_tensor("deg_t", [128, NWIN], f32, kind="ExternalInput")
    iot_in = nc.dram_tensor("iot", [128, 128], bf16, kind="ExternalInput")
    w0_in = nc.dram_tensor("w0", [F, PER_HOP], f32, kind="ExternalInput")
    w1_in = nc.dram_tensor("w1", [F, PER_HOP], f32, kind="ExternalInput")
    w2_in = nc.dram_tensor("w2", [F, PER_HOP], f32, kind="ExternalInput")
    wo_in = nc.dram_tensor("wout", [3 * PER_HOP, OUT], f32,
                           kind="ExternalInput")
    b0_in = nc.dram_tensor("b0", [PER_HOP, 1], f32, kind="ExternalInput")
    b1_in = nc.dram_tensor("b1", [PER_HOP, 1], f32, kind="ExternalInput")
    b2_in = nc.dram_tensor("b2", [PER_HOP, 1], f32, kind="ExternalInput")
    bo_in = nc.dram_tensor("bout", [OUT, 1], f32, kind="ExternalInput")
    out_t = nc.dram_tensor("out_t", [OUT, NPC], f32, kind="ExternalOutput")

    h1b = nc.dram_tensor("h1b", [NPC, F], bf16)
    h1f = nc.dram_tensor("h1f", [N, F], bf16, addr_space="Shared")

    def ts(w):
        return slice(w * WIN, (w + 1) * WIN)

    with tile.TileContext(nc) as tc:
        with (
            tc.tile_pool(name="persist", bufs=1) as pp,
            tc.tile_pool(name="gbuf", bufs=4) as gp,
            tc.tile_pool(name="work", bufs=4) as wp,
            tc.tile_pool(name="psum_y", bufs=2, space="PSUM") as psy,
            tc.tile_pool(name="psum_t", bufs=2, space="PSUM") as pst,
            tc.tile_pool(name="psum_d", bufs=2, space="PSUM") as psd,
        ):
            # ---- persistent loads ----
            idx_sb = pp.tile([128, int(IDXOFF[-1])], dt.int16)
            nc.sync.dma_start(out=idx_sb[:], in_=idx_in[:])
            tl_sb = pp.tile([128, int(TLOFF[-1])], bf16)
            nc.sync.dma_start(out=tl_sb[:], in_=tl_in[:])
            ds_sb = pp.tile([128, int(TLOFF[-1])], bf16)
            nc.sync.dma_start(out=ds_sb[:], in_=ds_in[:])
            iot_sb = pp.tile([128, 128], bf16)
            nc.sync.dma_start(out=iot_sb[:], in_=iot_in[:])
            deg_sb = pp.tile([128, NWIN], f32)
            nc.sync.dma_start(out=deg_sb[:], in_=deg_in[:])
            w0_sb = pp.tile([F, PER_HOP], f32)
            nc.sync.dma_start(out=w0_sb[:], in_=w0_in[:])
            w1_sb = pp.tile([F, PER_HOP], f32)
            nc.sync.dma_start(out=w1_sb[:], in_=w1_in[:])
            w2_sb = pp.tile([F, PER_HOP], f32)
            nc.sync.dma_start(out=w2_sb[:], in_=w2_in[:])
            wo_sb = []
            for k in range(3):
                t = pp.tile([PER_HOP, OUT], f32, tag=f"wo{k}")
                nc.sync.dma_start(
                    out=t[:], in_=wo_in.ap()[k * PER_HOP:(k + 1) * PER_HOP, :])
                wo_sb.append(t)
            b_sb = []
            for k, bin_ in enumerate((b0_in, b1_in, b2_in)):
                t = pp.tile([PER_HOP, 1], f32, tag=f"b{k}")
                nc.sync.dma_start(out=t[:], in_=bin_[:])
                b_sb.append(t)
            bo_sb = pp.tile([OUT, 1], f32)
            nc.sync.dma_start(out=bo_sb[:], in_=bo_in[:])
            ident = pp.tile([128, 128], f32)
            make_identity(nc, ident[:])
            iot_wide = pp.tile([128, NBTMAX * 128], bf16)
            for j in range(NBTMAX):
                nc.vector.tensor_copy(out=iot_wide[:, j * 128:(j + 1) * 128],
                                      in_=iot_sb[:])

            # dinv = 1/sqrt(deg); dinv2 = dinv^2  (both [128, NWIN])
            sq = pp.tile([128, NWIN], f32)
            nc.scalar.activation(out=sq[:], in_=deg_sb[:], func=AF.Sqrt)
            dinv = pp.tile([128, NWIN], f32)
            nc.vector.reciprocal(out=dinv[:], in_=sq[:])
            dinv2 = pp.tile([128, NWIN], f32)
            nc.vector.tensor_tensor(out=dinv2[:], in0=dinv[:], in1=dinv[:],
                                    op=ALU.mult)

            # ---- load x (window-major: [p, w*128+f] = x[w*128+p, f]) ----
            x_sb = pp.tile([128, NWIN * WIN], f32)
            nc.vector.memset(x_sb[:, (NWIN - 1) * WIN:], 0.0)
            nc.sync.dma_start(
                out=x_sb[:].rearrange("p (w f) -> p w f", f=F)[:, 0:NWIN - 1, :],
                in_=xo_in.ap()[0:NFULL, :].rearrange("(w p) f -> p w f", p=128),
            )
            nc.sync.dma_start(
                out=x_sb[0:NLAST, (NWIN - 1) * WIN:],
                in_=xo_in.ap()[NFULL:NPC, :],
            )

            # z_stage: hop1 self term dinv*x; overwritten to dinv^2*h1 later
            z_stage = pp.tile([128, NWIN * WIN], f32)
            for w in range(NWIN):
                nc.vector.tensor_scalar_mul(
                    out=z_stage[:, ts(w)], in0=x_sb[:, ts(w)],
                    scalar1=dinv[:, w:w + 1])

            h1_sb = pp.tile([128, NWIN * WIN], f32)
            qctr = [0]

            def head(w, h2_sb):
                relus = []
                for k, (h_sb, wk_sb) in enumerate(
                        ((x_sb, w0_sb), (h1_sb, w1_sb), (h2_sb, w2_sb))):
                    tp = pst.tile([128, 128], f32, tag="tp")
                    nc.tensor.transpose(out=tp[:], in_=h_sb[:, ts(w)],
                                        identity=ident[:])
                    hT = wp.tile([128, 128], f32, tag="hT")
                    nc.vector.tensor_copy(out=hT[:], in_=tp[:])
                    cps = psd.tile([PER_HOP, 128], f32, tag="cps")
                    nc.tensor.matmul(out=cps[:], lhsT=wk_sb[:], rhs=hT[:],
                                     start=True, stop=True)
                    rk = wp.tile([PER_HOP, 128], f32, tag=f"r{k}")
                    nc.scalar.activation(out=rk[:], in_=cps[:], func=AF.Relu,
                                         bias=b_sb[k][:])
                    relus.append(rk)
                ops = psd.tile([OUT, 128], f32, tag="ops")
                for k in range(3):
                    nc.tensor.matmul(out=ops[:], lhsT=wo_sb[k][:],
                                     rhs=relus[k][:],
                                     start=(k == 0), stop=(k == 2))
                ow = wp.tile([OUT, 128], f32, tag="ow")
                nc.scalar.activation(out=ow[:], in_=ops[:],
                                     func=AF.Identity, bias=bo_sb[:])
                lim = min(NPC, (w + 1) * WIN) - w * WIN
                nc.sync.dma_start(out=out_t.ap()[:, w * WIN:w * WIN + lim],
                                  in_=ow[:, 0:lim])

            def prop(table, dss, h_out, hop2):
                """One propagation sweep; hop2 also runs the head."""
                first4 = not hop2
                for w in range(NWIN):
                    NBT = NBTs[w]
                    g = gp.tile([128, NBTMAX * F], bf16, tag="g")
                    if first4 and w < 4:
                        # pool buffers start uninitialized; un-gathered tail
                        # slots must hold finite bf16 (S rows are 0 there)
                        nc.vector.memset(g[:], 0.0)
                    icol = int(IDXOFF[w])
                    blk = 0
                    for part, cgs, nbp in ((0, _chunks(GLs[w]), NBLs[w]),
                                           (1, _chunks(GHs[w]), NBHs[w])):
                        src = (table.ap()[0:SPLIT, :] if part == 0
                               else table.ap()[SPLIT:N, :])
                        done = 0          # idxs done within this part
                        for cg in cgs:
                            # chunk starts at a 128-slot boundary of the part
                            assert done % 128 == 0
                            nb_call = (cg * 16 + 127) // 128
                            b0 = blk + done // 128
                            nc.gpsimd.dma_gather(
                                out_ap=g[:, b0 * F:
                                         (b0 + nb_call) * F].rearrange(
                                    "p (b f) -> p b f", f=F),
                                in_ap=src,
                                idxs_ap=idx_sb[:, icol:icol + cg],
                                num_idxs=cg * 16, num_idxs_reg=cg * 16,
                                elem_size=F, queue_num=qctr[0] % 2)
                            qctr[0] += 1
                            icol += cg
                            done += cg * 16
                        blk += nbp
                    ps = psy.tile([128, F], f32)
                    c0, c1 = int(TLOFF[w]), int(TLOFF[w + 1])
                    sw = wp.tile([128, NBTMAX * 128], bf16, tag="s")
                    nc.vector.tensor_tensor(
                        out=sw[:, 0:NBT * 128].rearrange(
                            "p (b t) -> p b t", t=128),
                        in0=iot_wide[:, 0:NBT * 128].rearrange(
                            "p (b t) -> p b t", t=128),
                        in1=tl_sb[:, c0:c1].rearrange(
                            "p (b o) -> p b o", o=1).broadcast_to(
                            [128, NBT, 128]),
                        op=ALU.is_equal)
                    nc.vector.tensor_tensor(
                        out=sw[:, 0:NBT * 128].rearrange(
                            "p (b t) -> p b t", t=128),
                        in0=sw[:, 0:NBT * 128].rearrange(
                            "p (b t) -> p b t", t=128),
                        in1=dss[:, c0:c1].rearrange(
                            "p (b o) -> p b o", o=1).broadcast_to(
                            [128, NBT, 128]),
                        op=ALU.mult)
                    for j in range(NBT):
                        nc.tensor.matmul(
                            out=ps[:], lhsT=sw[:, j * 128:(j + 1) * 128],
                            rhs=g[:, j * F:(j + 1) * F],
                            start=(j == 0), stop=(j == NBT - 1))
                    # self loop term (z_stage), then h = dinv * ya
                    ya = wp.tile([128, F], f32, tag="ya")
                    nc.vector.tensor_tensor(
                        out=ya[:], in0=ps[:], in1=z_stage[:, ts(w)],
                        op=ALU.add)
                    nc.vector.tensor_scalar_mul(
                        out=h_out[:, ts(w)], in0=ya[:],
                        scalar1=dinv[:, w:w + 1])
                    if not hop2:
                        # stage hop-2 self term dinv*h1 (prop applies the
                        # remaining dinv_t); bounce h1 window
                        nc.vector.tensor_scalar_mul(
                            out=z_stage[:, ts(w)], in0=h_out[:, ts(w)],
                            scalar1=dinv[:, w:w + 1])
                        hb = wp.tile([128, F], bf16, tag="hb")
                        nc.vector.tensor_copy(out=hb[:], in_=h_out[:, ts(w)])
                        lim = min(NPC, (w + 1) * WIN) - w * WIN
                        nc.sync.dma_start(
                            out=h1b.ap()[w * WIN:w * WIN + lim, :],
                            in_=hb[0:lim, :])
                    else:
                        head(w, h_out)

            prop(x_in, ds_sb, h1_sb, hop2=False)
            nc.gpsimd.collective_compute(
                "AllGather", ALU.bypass,
                replica_groups=[list(range(NCORE))],
                ins=[h1b[:]], outs=[h1f[:]])

            # hop 2: same per-edge norm dinv_src * dinv_tgt (h1 is raw)
            h2_sb = pp.tile([128, NWIN * WIN], f32)
            prop(h1f, ds_sb, h2_sb, hop2=True)

    nc.compile()
    return nc


_CACHE = {}


def _get_nc(key):
    if key not in _CACHE:
        _CACHE[key] = _build(*key)
    return _CACHE[key]


def make_in_maps(x, pc, W0, b0, W1, b1, W2, b2, Wout, bout):
    iot = np.broadcast_to(
        np.arange(128, dtype=np.float32), (128, 128)).astype(ml_dtypes.bfloat16)
    x = np.ascontiguousarray(np.asarray(x, dtype=np.float32))
    common = {
        "iot": iot,
        "x_full": np.ascontiguousarray(x.astype(ml_dtypes.bfloat16)),
        "w0": np.asarray(W0, dtype=np.float32),
        "w1": np.asarray(W1, dtype=np.float32),
        "w2": np.asarray(W2, dtype=np.float32),
        "wout": np.asarray(Wout, dtype=np.float32),
        "b0": np.asarray(b0, dtype=np.float32).reshape(PER_HOP, 1),
        "b1": np.asarray(b1, dtype=np.float32).reshape(PER_HOP, 1),
        "b2": np.asarray(b2, dtype=np.float32).reshape(PER_HOP, 1),
        "bout": np.asarray(bout, dtype=np.float32).reshape(OUT, 1),
    }
    in_maps = []
    for c in range(NCORE):
        m = dict(common)
        m.update(pc[c])
        m["x_own"] = np.ascontiguousarray(x[c * NPC:(c + 1) * NPC])
        in_maps.append(m)
    return in_maps


def run(inputs, trace=False):
    from concourse.bass_utils import run_bass_kernel_spmd

    key, pc = _preprocess(np.asarray(inputs["edge_index"]))
    nc = _get_nc(key)
    in_maps = make_in_maps(
        inputs["x"], pc, inputs["W0"], inputs["b0"], inputs["W1"],
        inputs["b1"], inputs["W2"], inputs["b2"], inputs["Wout"],
        inputs["bout"])
    res = run_bass_kernel_spmd(nc, in_maps, core_ids=list(range(NCORE)),
                               trace=trace)
    out = np.empty((N, OUT), dtype=np.float32)
    for c in range(NCORE):
        out[c * NPC:(c + 1) * NPC] = np.asarray(res.results[c]["out_t"]).T
    return out, res


def kernel(x, edge_index, W0, b0, W1, b1, W2, b2, Wout, bout):
    out, _ = run({"x": x, "edge_index": edge_index, "W0": W0, "b0": b0,
                  "W1": W1, "b1": b1, "W2": W2, "b2": b2,
                  "Wout": Wout, "bout": bout})
    return out


# revision 18
# speedup vs baseline: 1.0980x; 1.0409x over previous
"""MixHop GNN kernel for one TRN2 chip (8 NeuronCores), Bass/Tile.

Math (matches the reference exactly):
    row/col = edge_index with self loops appended
    deg[t]  = #edges with col==t            (host: integer bincount)
    dinv    = 1/sqrt(deg)                   (device: sqrt + reciprocal)
    h1[t]   = dinv_t * (sum_{s->t} dinv_s * x_s  + dinv_t * x_t)
    h2[t]   = dinv_t * (sum_{s->t} dinv_s * h1_s + dinv_t * h1_t)
    out = relu(concat(x@W0+b0, h1@W1+b1, h2@W2+b2)) @ Wout + bout

Sharding: core c owns target nodes [c*N/8, (c+1)*N/8). Edges (self loops
excluded -- those enter via the z_stage add, since the needed value is
resident) are bucketed by target into windows of 128 consecutive
targets, split by source (< 32768 vs >=, the int16 limit of dma_gather),
each part padded to blocks of 128 (uniform across cores -> one SPMD
program). Per window: dma_gather pulls source rows straight from the
raw x table (hop 1) / the AllGathered h1 table (hop 2) -- the source-
side norm factor is folded into the selection matrix S built with ONE
fused DVE op: S = (iota == tl) * dinv_src,
and a PE matmul S.T @ G accumulates the scaled segment-sum in PSUM.
Gather descriptor generation is the bottleneck engine (GpSimd SWDGE,
~8.4 ns/idx on one queue), so consecutive gather calls alternate
between SWDGE queues 0/1, which overlap generation (~5.9 ns/idx).
Only ONE collective remains (AllGather of h1); the hop-1 table is the
raw x input. The dense head is interleaved into the hop-2 window loop.
"""
import numpy as np
import ml_dtypes

N = 50000
F = 128
NCORE = 8
NPC = N // NCORE          # 6250 nodes per core
WIN = 128                 # targets per window
NWIN = (NPC + WIN - 1) // WIN   # 49 (48 full + 1 partial of 106)
PER_HOP = 64
OUT = 64
SPLIT = 32768             # int16 index limit for dma_gather tables
MAXBLK = 8                # max 1024 idxs per dma_gather call
PAD_TL = 300.0            # dummy-edge tl: matches no iota value -> zero S row


MAXG = 64                 # 16-idx groups per call (64 -> 1024 idxs)


def _chunks(ng):
    """Split ng 16-idx groups into calls <= MAXG; all but the last call
    must cover a multiple of 128 idxs (8 groups) so each call starts at a
    128-slot boundary of the gather output."""
    if ng <= 0:
        return []
    k = (ng + MAXG - 1) // MAXG
    out = []
    left = ng
    for i in range(k - 1):
        c = min(MAXG, max(8, ((left // (k - i)) + 7) // 8 * 8))
        out.append(c)
        left -= c
    out.append(left)
    return out


def _preprocess(edge_index):
    """Bucket edges by (core, target-window, source-half); pad uniformly.

    Returns (NBL, NBH, per_core list of dicts with idx16, tl_t, ds_t
    (dinv_src per slot), deg_t).
    """
    row = np.asarray(edge_index[0], dtype=np.int64)
    col = np.asarray(edge_index[1], dtype=np.int64)
    deg = (np.bincount(col, minlength=N) + 1).astype(np.float64)
    dinv = (1.0 / np.sqrt(deg)).astype(np.float32)

    cores = []
    for c in range(NCORE):
        lo, hi = c * NPC, (c + 1) * NPC
        sel = (col >= lo) & (col < hi)
        r = row[sel]
        t = col[sel] - lo
        order = np.argsort(t, kind="stable")
        r, t = r[order], t[order]
        is_lo = r < SPLIT
        ra = r
        rb = r - SPLIT
        parts = []
        for w in range(NWIN):
            wsel = (t // WIN) == w
            rl = ra[wsel & is_lo]
            tl_l = (t[wsel & is_lo] % WIN).astype(np.float32)
            dl = r[wsel & is_lo]
            rh = rb[wsel & ~is_lo]
            tl_h = (t[wsel & ~is_lo] % WIN).astype(np.float32)
            dh = r[wsel & ~is_lo]
            parts.append((rl, tl_l, dl, rh, tl_h, dh))
        cores.append(parts)

    # real max counts (over cores), rounded to 16 (idx wrap granularity)
    GLs = tuple(int(-(-max(len(cores[c][w][0]) for c in range(NCORE)) // 16))
                for w in range(NWIN))
    GHs = tuple(int(-(-max(len(cores[c][w][3]) for c in range(NCORE)) // 16))
                for w in range(NWIN))
    NBLs = tuple((g * 16 + 127) // 128 for g in GLs)
    NBHs = tuple((g * 16 + 127) // 128 for g in GHs)
    IDXOFF = np.concatenate(
        [[0], np.cumsum([gl + gh for gl, gh in zip(GLs, GHs)])]).astype(int)
    TLOFF = np.concatenate(
        [[0], np.cumsum([nl + nh for nl, nh in zip(NBLs, NBHs)])]).astype(int)
    out = []
    for ci, parts in enumerate(cores):
        idx16 = np.zeros((128, int(IDXOFF[-1])), dtype=np.int16)
        tl_t = np.full((128, int(TLOFF[-1])), PAD_TL, dtype=np.float32)
        ds_t = np.zeros((128, int(TLOFF[-1])), dtype=np.float32)
        for w, (rl, tl_l, dl, rh, tl_h, dh) in enumerate(parts):
            nbl_w, nbh_w = NBLs[w], NBHs[w]
            nbt_w = nbl_w + nbh_w
            gl_w, gh_w = GLs[w], GHs[w]
            lo_chunks, hi_chunks = _chunks(gl_w), _chunks(gh_w)
            ilo = np.zeros(gl_w * 16, dtype=np.int64)
            ilo[:len(rl)] = rl
            ihi = np.zeros(gh_w * 16, dtype=np.int64)
            ihi[:len(rh)] = rh
            # tl / dinv_src streams: [lo blocks..., hi blocks...]
            tw = np.full(nbt_w * 128, PAD_TL, dtype=np.float32)
            tw[:len(tl_l)] = tl_l
            tw[nbl_w * 128:nbl_w * 128 + len(tl_h)] = tl_h
            tl_t[:, TLOFF[w]:TLOFF[w + 1]] = tw.reshape(nbt_w, 128).T
            dw = np.zeros(nbt_w * 128, dtype=np.float32)
            dw[:len(dl)] = dinv[dl]
            dw[nbl_w * 128:nbl_w * 128 + len(dh)] = dinv[dh]
            ds_t[:, TLOFF[w]:TLOFF[w + 1]] = dw.reshape(nbt_w, 128).T
            # idx stream: per call, [16-wrap then replicate x8 partitions]
            cols = []
            for chunks, arr in ((lo_chunks, ilo), (hi_chunks, ihi)):
                off = 0
                for cg in chunks:
                    a = arr[off:off + cg * 16].astype(np.int16)
                    cols.append(np.tile(a.reshape(-1, 16).T, (8, 1)))
                    off += cg * 16
            if cols:
                idx16[:, IDXOFF[w]:IDXOFF[w + 1]] = np.concatenate(
                    cols, axis=1)
        lo = ci * NPC
        dpad = np.ones(NWIN * WIN, dtype=np.float32)
        dpad[:NPC] = deg[lo:lo + NPC]
        out.append({"idx16": np.ascontiguousarray(idx16),
                    "tl_t": np.ascontiguousarray(
                        tl_t.astype(ml_dtypes.bfloat16)),
                    "ds_t": np.ascontiguousarray(
                        ds_t.astype(ml_dtypes.bfloat16)),
                    "deg_t": np.ascontiguousarray(
                        dpad.reshape(NWIN, WIN).T.astype(np.float32))})
    return (NBLs, NBHs, GLs, GHs), out


def _build(NBLs, NBHs, GLs, GHs):
    import concourse.bass as bass  # noqa: F401
    import concourse.bacc as bacc
    import concourse.tile as tile
    import concourse.mybir as mybir
    from concourse.masks import make_identity

    dt = mybir.dt
    f32 = dt.float32
    bf16 = dt.bfloat16
    AF = mybir.ActivationFunctionType
    ALU = mybir.AluOpType
    NBTs = [nl + nh for nl, nh in zip(NBLs, NBHs)]
    NBTMAX = max(NBTs)
    IDXOFF = np.concatenate(
        [[0], np.cumsum([gl + gh for gl, gh in zip(GLs, GHs)])]).astype(int)
    TLOFF = np.concatenate([[0], np.cumsum(NBTs)]).astype(int)
    NFULL = (NWIN - 1) * WIN                 # 6144 rows in full windows
    NLAST = NPC - NFULL                      # 106 rows in the partial window

    nc = bacc.Bacc("TRN2", target_bir_lowering=False, debug=False,
                   num_devices=NCORE, num_swdge_queues=2)

    x_in = nc.dram_tensor("x_full", [N, F], bf16, kind="ExternalInput")
    xo_in = nc.dram_tensor("x_own", [NPC, F], f32, kind="ExternalInput")
    idx_in = nc.dram_tensor("idx16", [128, int(IDXOFF[-1])], dt.int16,
                            kind="ExternalInput")
    tl_in = nc.dram_tensor("tl_t", [128, int(TLOFF[-1])], bf16,
                           kind="ExternalInput")
    ds_in = nc.dram_tensor("ds_t", [128, int(TLOFF[-1])], bf16,
                           kind="ExternalInput")
    deg_in = nc.dram_tensor("deg_t", [128, NWIN], f32, kind="ExternalInput")
    iot_in = nc.dram_tensor("iot", [128, 128], bf16, kind="ExternalInput")
    w0_in = nc.dram_tensor("w0", [F, PER_HOP], f32, kind="ExternalInput")
    w1_in = nc.dram_tensor("w1", [F, PER_HOP], f32, kind="ExternalInput")
    w2_in = nc.dram_tensor("w2", [F, PER_HOP], f32, kind="ExternalInput")
    wo_in = nc.dram_tensor("wout", [3 * PER_HOP, OUT], f32,
                           kind="ExternalInput")
    b0_in = nc.dram_tensor("b0", [PER_HOP, 1], f32, kind="ExternalInput")
    b1_in = nc.dram_tensor("b1", [PER_HOP, 1], f32, kind="ExternalInput")
    b2_in = nc.dram_tensor("b2", [PER_HOP, 1], f32, kind="ExternalInput")
    bo_in = nc.dram_tensor("bout", [OUT, 1], f32, kind="ExternalInput")
    out_t = nc.dram_tensor("out_t", [OUT, NPC], f32, kind="ExternalOutput")

    h1b = nc.dram_tensor("h1b", [NPC, F], bf16)
    h1f = nc.dram_tensor("h1f", [N, F], bf16, addr_space="Shared")

    def ts(w):
        return slice(w * WIN, (w + 1) * WIN)

    with tile.TileContext(nc) as tc:
        with (
            tc.tile_pool(name="persist", bufs=1) as pp,
            tc.tile_pool(name="gbuf", bufs=6) as gp,
            tc.tile_pool(name="work", bufs=6) as wp,
            tc.tile_pool(name="psum_y", bufs=2, space="PSUM") as psy,
            tc.tile_pool(name="psum_t", bufs=2, space="PSUM") as pst,
            tc.tile_pool(name="psum_d", bufs=2, space="PSUM") as psd,
        ):
            # ---- persistent loads ----
            idx_sb = pp.tile([128, int(IDXOFF[-1])], dt.int16)
            nc.sync.dma_start(out=idx_sb[:], in_=idx_in[:])
            tl_sb = pp.tile([128, int(TLOFF[-1])], bf16)
            nc.sync.dma_start(out=tl_sb[:], in_=tl_in[:])
            ds_sb = pp.tile([128, int(TLOFF[-1])], bf16)
            nc.sync.dma_start(out=ds_sb[:], in_=ds_in[:])
            iot_sb = pp.tile([128, 128], bf16)
            nc.sync.dma_start(out=iot_sb[:], in_=iot_in[:])
            deg_sb = pp.tile([128, NWIN], f32)
            nc.sync.dma_start(out=deg_sb[:], in_=deg_in[:])
            w0_sb = pp.tile([F, PER_HOP], f32)
            nc.sync.dma_start(out=w0_sb[:], in_=w0_in[:])
            w1_sb = pp.tile([F, PER_HOP], f32)
            nc.sync.dma_start(out=w1_sb[:], in_=w1_in[:])
            w2_sb = pp.tile([F, PER_HOP], f32)
            nc.sync.dma_start(out=w2_sb[:], in_=w2_in[:])
            wo_sb = []
            for k in range(3):
                t = pp.tile([PER_HOP, OUT], f32, tag=f"wo{k}")
                nc.sync.dma_start(
                    out=t[:], in_=wo_in.ap()[k * PER_HOP:(k + 1) * PER_HOP, :])
                wo_sb.append(t)
            b_sb = []
            for k, bin_ in enumerate((b0_in, b1_in, b2_in)):
                t = pp.tile([PER_HOP, 1], f32, tag=f"b{k}")
                nc.sync.dma_start(out=t[:], in_=bin_[:])
                b_sb.append(t)
            bo_sb = pp.tile([OUT, 1], f32)
            nc.sync.dma_start(out=bo_sb[:], in_=bo_in[:])
            ident = pp.tile([128, 128], f32)
            make_identity(nc, ident[:])
            iot_wide = pp.tile([128, NBTMAX * 128], bf16)
            for j in range(NBTMAX):
                nc.vector.tensor_copy(out=iot_wide[:, j * 128:(j + 1) * 128],
                                      in_=iot_sb[:])

            # dinv = 1/sqrt(deg); dinv2 = dinv^2  (both [128, NWIN])
            sq = pp.tile([128, NWIN], f32)
            nc.scalar.activation(out=sq[:], in_=deg_sb[:], func=AF.Sqrt)
            dinv = pp.tile([128, NWIN], f32)
            nc.vector.reciprocal(out=dinv[:], in_=sq[:])
            dinv2 = pp.tile([128, NWIN], f32)
            nc.vector.tensor_tensor(out=dinv2[:], in0=dinv[:], in1=dinv[:],
                                    op=ALU.mult)

            # ---- load x (window-major: [p, w*128+f] = x[w*128+p, f]) ----
            x_sb = pp.tile([128, NWIN * WIN], f32)
            nc.vector.memset(x_sb[:, (NWIN - 1) * WIN:], 0.0)
            nc.sync.dma_start(
                out=x_sb[:].rearrange("p (w f) -> p w f", f=F)[:, 0:NWIN - 1, :],
                in_=xo_in.ap()[0:NFULL, :].rearrange("(w p) f -> p w f", p=128),
            )
            nc.sync.dma_start(
                out=x_sb[0:NLAST, (NWIN - 1) * WIN:],
                in_=xo_in.ap()[NFULL:NPC, :],
            )

            # z_stage: hop1 self term dinv*x (filled lazily in the loop);
            # overwritten to dinv*h1 for hop 2
            z_stage = pp.tile([128, NWIN * WIN], f32)

            h1_sb = pp.tile([128, NWIN * WIN], f32)
            qctr = [0]

            def head(w, h2_sb):
                relus = []
                for k, (h_sb, wk_sb) in enumerate(
                        ((x_sb, w0_sb), (h1_sb, w1_sb), (h2_sb, w2_sb))):
                    tp = pst.tile([128, 128], f32, tag="tp")
                    nc.tensor.transpose(out=tp[:], in_=h_sb[:, ts(w)],
                                        identity=ident[:])
                    hT = wp.tile([128, 128], f32, tag="hT")
                    nc.vector.tensor_copy(out=hT[:], in_=tp[:])
                    cps = psd.tile([PER_HOP, 128], f32, tag="cps")
                    nc.tensor.matmul(out=cps[:], lhsT=wk_sb[:], rhs=hT[:],
                                     start=True, stop=True)
                    rk = wp.tile([PER_HOP, 128], f32, tag=f"r{k}")
                    nc.scalar.activation(out=rk[:], in_=cps[:], func=AF.Relu,
                                         bias=b_sb[k][:])
                    relus.append(rk)
                ops = psd.tile([OUT, 128], f32, tag="ops")
                for k in range(3):
                    nc.tensor.matmul(out=ops[:], lhsT=wo_sb[k][:],
                                     rhs=relus[k][:],
                                     start=(k == 0), stop=(k == 2))
                ow = wp.tile([OUT, 128], f32, tag="ow")
                nc.scalar.activation(out=ow[:], in_=ops[:],
                                     func=AF.Identity, bias=bo_sb[:])
                lim = min(NPC, (w + 1) * WIN) - w * WIN
                nc.sync.dma_start(out=out_t.ap()[:, w * WIN:w * WIN + lim],
                                  in_=ow[:, 0:lim])

            def prop(table, dss, h_out, hop2):
                """One propagation sweep; hop2 also runs the head."""
                first4 = not hop2
                for w in range(NWIN):
                    NBT = NBTs[w]
                    g = gp.tile([128, NBTMAX * F], bf16, tag="g")
                    if first4 and w < 6:
                        # pool buffers start uninitialized; un-gathered tail
                        # slots must hold finite bf16 (S rows are 0 there)
                        nc.vector.memset(g[:], 0.0)
                    icol = int(IDXOFF[w])
                    blk = 0
                    for part, cgs, nbp in ((0, _chunks(GLs[w]), NBLs[w]),
                                           (1, _chunks(GHs[w]), NBHs[w])):
                        src = (table.ap()[0:SPLIT, :] if part == 0
                               else table.ap()[SPLIT:N, :])
                        done = 0          # idxs done within this part
                        for cg in cgs:
                            # chunk starts at a 128-slot boundary of the part
                            assert done % 128 == 0
                            nb_call = (cg * 16 + 127) // 128
                            b0 = blk + done // 128
                            nc.gpsimd.dma_gather(
                                out_ap=g[:, b0 * F:
                                         (b0 + nb_call) * F].rearrange(
                                    "p (b f) -> p b f", f=F),
                                in_ap=src,
                                idxs_ap=idx_sb[:, icol:icol + cg],
                                num_idxs=cg * 16, num_idxs_reg=cg * 16,
                                elem_size=F, queue_num=qctr[0] % 2)
                            qctr[0] += 1
                            icol += cg
                            done += cg * 16
                        blk += nbp
                    ps = psy.tile([128, F], f32)
                    c0, c1 = int(TLOFF[w]), int(TLOFF[w + 1])
                    sw = wp.tile([128, NBTMAX * 128], bf16, tag="s")
                    nc.vector.tensor_tensor(
                        out=sw[:, 0:NBT * 128].rearrange(
                            "p (b t) -> p b t", t=128),
                        in0=iot_wide[:, 0:NBT * 128].rearrange(
                            "p (b t) -> p b t", t=128),
                        in1=tl_sb[:, c0:c1].rearrange(
                            "p (b o) -> p b o", o=1).broadcast_to(
                            [128, NBT, 128]),
                        op=ALU.is_equal)
                    nc.vector.tensor_tensor(
                        out=sw[:, 0:NBT * 128].rearrange(
                            "p (b t) -> p b t", t=128),
                        in0=sw[:, 0:NBT * 128].rearrange(
                            "p (b t) -> p b t", t=128),
                        in1=dss[:, c0:c1].rearrange(
                            "p (b o) -> p b o", o=1).broadcast_to(
                            [128, NBT, 128]),
                        op=ALU.mult)
                    for j in range(NBT):
                        nc.tensor.matmul(
                            out=ps[:], lhsT=sw[:, j * 128:(j + 1) * 128],
                            rhs=g[:, j * F:(j + 1) * F],
                            start=(j == 0), stop=(j == NBT - 1))
                    # self loop term (z_stage), then h = dinv * ya
                    if not hop2:
                        nc.vector.tensor_scalar_mul(
                            out=z_stage[:, ts(w)], in0=x_sb[:, ts(w)],
                            scalar1=dinv[:, w:w + 1])
                    ya = wp.tile([128, F], f32, tag="ya")
                    nc.vector.tensor_tensor(
                        out=ya[:], in0=ps[:], in1=z_stage[:, ts(w)],
                        op=ALU.add)
                    nc.vector.tensor_scalar_mul(
                        out=h_out[:, ts(w)], in0=ya[:],
                        scalar1=dinv[:, w:w + 1])
                    if not hop2:
                        # stage hop-2 self term dinv*h1 (prop applies the
                        # remaining dinv_t); bounce h1 window
                        nc.vector.tensor_scalar_mul(
                            out=z_stage[:, ts(w)], in0=h_out[:, ts(w)],
                            scalar1=dinv[:, w:w + 1])
                        hb = wp.tile([128, F], bf16, tag="hb")
                        nc.vector.tensor_copy(out=hb[:], in_=h_out[:, ts(w)])
                        lim = min(NPC, (w + 1) * WIN) - w * WIN
                        nc.sync.dma_start(
                            out=h1b.ap()[w * WIN:w * WIN + lim, :],
                            in_=hb[0:lim, :])
                    else:
                        head(w, h_out)

            prop(x_in, ds_sb, h1_sb, hop2=False)
            nc.gpsimd.collective_compute(
                "AllGather", ALU.bypass,
                replica_groups=[list(range(NCORE))],
                ins=[h1b[:]], outs=[h1f[:]])

            # hop 2: same per-edge norm dinv_src * dinv_tgt (h1 is raw)
            h2_sb = pp.tile([128, NWIN * WIN], f32)
            prop(h1f, ds_sb, h2_sb, hop2=True)

    nc.compile()
    return nc


_CACHE = {}


def _get_nc(key):
    if key not in _CACHE:
        _CACHE[key] = _build(*key)
    return _CACHE[key]


def make_in_maps(x, pc, W0, b0, W1, b1, W2, b2, Wout, bout):
    iot = np.broadcast_to(
        np.arange(128, dtype=np.float32), (128, 128)).astype(ml_dtypes.bfloat16)
    x = np.ascontiguousarray(np.asarray(x, dtype=np.float32))
    common = {
        "iot": iot,
        "x_full": np.ascontiguousarray(x.astype(ml_dtypes.bfloat16)),
        "w0": np.asarray(W0, dtype=np.float32),
        "w1": np.asarray(W1, dtype=np.float32),
        "w2": np.asarray(W2, dtype=np.float32),
        "wout": np.asarray(Wout, dtype=np.float32),
        "b0": np.asarray(b0, dtype=np.float32).reshape(PER_HOP, 1),
        "b1": np.asarray(b1, dtype=np.float32).reshape(PER_HOP, 1),
        "b2": np.asarray(b2, dtype=np.float32).reshape(PER_HOP, 1),
        "bout": np.asarray(bout, dtype=np.float32).reshape(OUT, 1),
    }
    in_maps = []
    for c in range(NCORE):
        m = dict(common)
        m.update(pc[c])
        m["x_own"] = np.ascontiguousarray(x[c * NPC:(c + 1) * NPC])
        in_maps.append(m)
    return in_maps


def run(inputs, trace=False):
    from concourse.bass_utils import run_bass_kernel_spmd

    key, pc = _preprocess(np.asarray(inputs["edge_index"]))
    nc = _get_nc(key)
    in_maps = make_in_maps(
        inputs["x"], pc, inputs["W0"], inputs["b0"], inputs["W1"],
        inputs["b1"], inputs["W2"], inputs["b2"], inputs["Wout"],
        inputs["bout"])
    res = run_bass_kernel_spmd(nc, in_maps, core_ids=list(range(NCORE)),
                               trace=trace)
    out = np.empty((N, OUT), dtype=np.float32)
    for c in range(NCORE):
        out[c * NPC:(c + 1) * NPC] = np.asarray(res.results[c]["out_t"]).T
    return out, res


def kernel(x, edge_index, W0, b0, W1, b1, W2, b2, Wout, bout):
    out, _ = run({"x": x, "edge_index": edge_index, "W0": W0, "b0": b0,
                  "W1": W1, "b1": b1, "W2": W2, "b2": b2,
                  "Wout": Wout, "bout": bout})
    return out
